# revision 18
# baseline (speedup 1.0000x reference)
"""Causal self-attention (B=4, T=2048, C=512, H=8, D=64) on 8 TRN2 NeuronCores.

Sharding: core = (batch b, head-group hg) with 4 batches x 2 head groups of 4
heads.  Each core computes q/k/v projections for its 4 heads, causal
attention, and a partial output projection (its 256 rows of W_out); the host
sums the two head-group partials per batch.

Per-core kernel layout notes:
  - x is fed pre-transposed ([C, T]) so all projection matmuls stream
    contiguously (contraction dim = partitions).
  - Attention computes S^T blocks ([tk, tq]) directly by swapping matmul
    operands, so no on-chip transposes are needed anywhere.  tq widths are
    ragged (only tq >= tk is computed), so no masking of above-diagonal
    blocks is needed either -- just a triu mask on the diagonal block.
  - Softmax denominators come from a ones-column appended to V (row 64 of the
    PV accumulator); normalization happens on the [64, 512] O^T eviction via
    a gpsimd partition_broadcast of the reciprocal row.
  - Matmul operands are bf16 (inputs rounded host-side); accumulation and the
    softmax arithmetic stay fp32 in PSUM.
"""

import os
from contextlib import ExitStack

import numpy as np
import ml_dtypes

import concourse.bass as bass
import concourse.tile as tile
from concourse import bacc, mybir
from concourse.bass import ts, ds
from concourse.bass_utils import run_bass_kernel_spmd
from concourse.masks import make_upper_triangular

# Problem constants (hardcoded per harness contract).
B = 4
T = 2048
C = 512
H = 8
D = 64
HG = 2                 # head groups (tensor-parallel dim)
HPC = H // HG          # heads per core = 4
M = HPC * D            # local head width = 256
P = 128
NT = T // P            # 16 t-tiles
NS = T // 512          # 4 t-supertiles
KC = C // P            # 4 contraction chunks of x
F32 = mybir.dt.float32
BF16 = mybir.dt.bfloat16

_LAST_RESULTS = None   # stashed BassKernelResults for test harness inspection


def build_attention_kernel():
    nc = bacc.Bacc("TRN2", target_bir_lowering=False, debug=False, num_devices=B * HG)

    xT = nc.dram_tensor("xT", [C, T], BF16, kind="ExternalInput").ap()
    wq = nc.dram_tensor("wq", [C, M], BF16, kind="ExternalInput").ap()
    wk = nc.dram_tensor("wk", [C, M], BF16, kind="ExternalInput").ap()
    wv = nc.dram_tensor("wv", [C, M], BF16, kind="ExternalInput").ap()
    wo = nc.dram_tensor("wo", [M, C], BF16, kind="ExternalInput").ap()
    y = nc.dram_tensor("y", [T, C], F32, kind="ExternalOutput").ap()

    with tile.TileContext(nc) as tc:
        with ExitStack() as ctx:
            emit_kernel(ctx, tc, xT, wq, wk, wv, wo, y)
    nc.compile()
    return nc


def emit_kernel(ctx, tc, xT, wq, wk, wv, wo, y):
    nc = tc.nc
    Exp = mybir.ActivationFunctionType.Exp
    scale = 1.0 / np.sqrt(D)

    const = ctx.enter_context(tc.tile_pool(name="const", bufs=1))
    xt_pool = ctx.enter_context(tc.tile_pool(name="xt", bufs=1))
    w_pool = ctx.enter_context(tc.tile_pool(name="w", bufs=1))
    qkv_pool = ctx.enter_context(tc.tile_pool(name="qkv", bufs=1))
    pt_pool = ctx.enter_context(tc.tile_pool(name="pt", bufs=6))
    ot_pool = ctx.enter_context(tc.tile_pool(name="ot", bufs=1))
    ysb_pool = ctx.enter_context(tc.tile_pool(name="ysb", bufs=3))
    small_pool = ctx.enter_context(tc.tile_pool(name="small", bufs=4))
    psum_s = ctx.enter_context(tc.tile_pool(name="psum_s", bufs=2, space="PSUM"))
    psum_ot = ctx.enter_context(tc.tile_pool(name="psum_ot", bufs=2, space="PSUM"))

    # --- constants ---
    triu_f32 = const.tile([P, P], F32)
    make_upper_triangular(nc, triu_f32[:], val=1.0, diag=True)
    triu = const.tile([P, P], BF16)
    nc.vector.tensor_copy(triu[:], triu_f32[:])

    # --- load weights (single strided DMA each) and xT ---
    wq_sb = w_pool.tile([P, KC, M], BF16)
    nc.sync.dma_start(wq_sb[:], wq.rearrange("(kc p) m -> p kc m", p=P))
    wk_sb = w_pool.tile([P, KC, M], BF16)
    nc.sync.dma_start(wk_sb[:], wk.rearrange("(kc p) m -> p kc m", p=P))
    wv_sb = w_pool.tile([P, KC, M], BF16)
    nc.sync.dma_start(wv_sb[:], wv.rearrange("(kc p) m -> p kc m", p=P))
    wo_sb = w_pool.tile([P, M // P, C], BF16)
    nc.sync.dma_start(wo_sb[:], wo.rearrange("(kc p) n -> p kc n", p=P))

    xt_sb = []
    for kc in range(KC):
        t_ = xt_pool.tile([P, T], BF16, name=f"xt{kc}")
        xt_sb.append(t_)
    for tc_ in range(NS):
        for kc in range(KC):
            nc.sync.dma_start(
                xt_sb[kc][:, ts(tc_, 512)], xT[ts(kc, P), ts(tc_, 512)]
            )

    # --- QKV projections ---
    # Q^T, K^T: [M, T] as 2 partition-tiles of [128, T] (2 heads each).
    qt_sb = [qkv_pool.tile([P, T], BF16, name=f"qt{i}") for i in range(M // P)]
    kt_sb = [qkv_pool.tile([P, T], BF16, name=f"kt{i}") for i in range(M // P)]
    # V (+ones col): [128, NT, HPC, D+1]; V block tt rows t in tile, per head.
    v_sb = qkv_pool.tile([P, NT, HPC, D + 1], BF16)
    nc.gpsimd.memset(v_sb[:, :, :, D : D + 1], 1.0)
    def emit_v_block(tt):
        s_ps = psum_s.tile([P, 2, 512], F32, name="s_ps")
        ps = s_ps[:, 0, 0:M]
        for kc in range(KC):
            nc.tensor.matmul(
                ps,
                xt_sb[kc][:, ts(tt, P)],
                wv_sb[:, kc, :],
                start=(kc == 0),
                stop=(kc == KC - 1),
            )
        nc.vector.tensor_copy(
            v_sb[:, tt, :, 0:D], ps.rearrange("p (h d) -> p h d", d=D)
        )

    for tc_ in range(NS):
        for mo in range(M // P):
            for w_sb, dst in ((wq_sb, qt_sb), (wk_sb, kt_sb)):
                s_ps = psum_s.tile([P, 2, 512], F32, name="s_ps")
                ps = s_ps[:, 0, :]
                for kc in range(KC):
                    nc.tensor.matmul(
                        ps,
                        w_sb[:, kc, ts(mo, P)],
                        xt_sb[kc][:, ts(tc_, 512)],
                        start=(kc == 0),
                        stop=(kc == KC - 1),
                    )
                nc.scalar.copy(dst[mo][:, ts(tc_, 512)], ps)
        for tt in range(4 * tc_, 4 * tc_ + 4):
            emit_v_block(tt)

    # V (+ones col): [128, NT, HPC, D+1]; V block tt rows t in tile, per head.

    # --- attention + output projection, supertile by supertile ---
    # Heads are processed in pairs (partition offsets 0 and 64 of one qt/kt
    # tile): the two S^T matmuls are 64x128 row-tiled (T0/T8) and can overlap
    # on the PE array; exp covers both heads in one strided ACT op.
    ot_sb = [ot_pool.tile([P, T], BF16, name=f"ot{i}") for i in range(M // P)]

    def emit_yproj(tt, evict_engine):
        s_ps = psum_s.tile([P, 2, 512], F32, name="s_ps")
        ps = s_ps[:, 0, :]
        for mo in range(M // P):
            nc.tensor.matmul(
                ps,
                ot_sb[mo][:, ts(tt, P)],
                wo_sb[:, mo, :],
                start=(mo == 0),
                stop=(mo == M // P - 1),
            )
        y_sb = ysb_pool.tile([P, C], F32)
        evict_engine.copy(y_sb[:], ps) if evict_engine is nc.scalar \
            else evict_engine.tensor_copy(y_sb[:], ps)
        nc.sync.dma_start(y[ts(tt, P), :], y_sb[:])

    pending_yproj = []
    for s in range(NS):
        nblk = 4 * (s + 1)
        stride = max(1, nblk // 5)
        ot_units = [
            psum_ot.tile([D + 1, 2, 512], F32, name="ot_ps")
            for _ in range(M // P)
        ]
        for j in range(nblk):
            # ragged tq range: only tq >= tk (block-rounded)
            off = max(0, j - 4 * s) * P
            n = 512 - off
            for mo in range(M // P):
                s_ps = psum_s.tile([P, 2, 512], F32, name="s_ps")
                for hs, po in ((0, 0), (1, D)):
                    nc.tensor.matmul(
                        s_ps[:, hs, 0:n],
                        kt_sb[mo][ds(po, D), ts(j, P)],
                        qt_sb[mo][ds(po, D), ds(512 * s + off, n)],
                        start=True,
                        stop=True,
                    )
                pt = pt_pool.tile([P, 2, 512], BF16)
                nc.scalar.activation(
                    pt[:, :, 0:n], s_ps[:, :, 0:n], Exp, scale=scale
                )
                if off > 0 or j == 4 * s:
                    # first 128 cols of the ragged region are the diagonal blk
                    for hs in (0, 1):
                        nc.vector.tensor_mul(
                            pt[:, hs, 0:P], pt[:, hs, 0:P], triu[:]
                        )
                for hs, h in ((0, 2 * mo), (1, 2 * mo + 1)):
                    nc.tensor.matmul(
                        ot_units[mo][:, hs, ds(off, n)],
                        v_sb[:, j, h, :],
                        pt[:, hs, ds(0, n)],
                        start=(j == 0),
                        stop=(j == nblk - 1),
                    )
            # hide the previous supertile's output projections in this
            # supertile's attention stream
            if pending_yproj and j % stride == stride - 1:
                emit_yproj(pending_yproj.pop(0), nc.vector)
        for mo in range(M // P):
            ot_ps = ot_units[mo]
            sums = small_pool.tile([1, 2, 512], F32)
            nc.vector.tensor_copy(sums[:], ot_ps[ds(D, 1), :, :])
            recip = small_pool.tile([1, 2, 512], F32)
            nc.vector.reciprocal_approx_fast(recip[:], sums[:])
            bcast = small_pool.tile([D, 2, 512], F32)
            nc.gpsimd.partition_broadcast(bcast[:], recip[:])
            for hs, po in ((0, 0), (1, D)):
                nc.vector.tensor_mul(
                    ot_sb[mo][ds(po, D), ts(s, 512)],
                    ot_ps[0:D, hs, :],
                    bcast[:, hs, :],
                )
        pending_yproj.extend(range(4 * s, 4 * s + 4))

    # tail: whatever projections are still pending (last supertile's)
    for i, tt in enumerate(pending_yproj):
        emit_yproj(tt, nc.scalar if i % 2 == 0 else nc.vector)


def shard_inputs(x, W_qkv, W_out):
    """Full inputs -> list of 8 per-core input dicts (core = b*HG + hg)."""
    bf16 = ml_dtypes.bfloat16
    x = np.asarray(x, dtype=np.float32)
    W_qkv = np.asarray(W_qkv, dtype=np.float32).astype(bf16)
    W_out = np.asarray(W_out, dtype=np.float32).astype(bf16)
    in_maps = []
    for b in range(B):
        xT = np.ascontiguousarray(x[b].T).astype(bf16)
        for hg in range(HG):
            cols = slice(hg * M, (hg + 1) * M)
            in_maps.append(
                {
                    "xT": xT,
                    "wq": np.ascontiguousarray(W_qkv[:, 0 * C :][:, cols]),
                    "wk": np.ascontiguousarray(W_qkv[:, 1 * C :][:, cols]),
                    "wv": np.ascontiguousarray(W_qkv[:, 2 * C :][:, cols]),
                    "wo": np.ascontiguousarray(W_out[hg * M : (hg + 1) * M, :]),
                }
            )
    return in_maps


_NC_CACHE = None


def kernel(x, W_qkv, W_out):
    global _NC_CACHE, _LAST_RESULTS
    if _NC_CACHE is None:
        _NC_CACHE = build_attention_kernel()
    nc = _NC_CACHE
    in_maps = shard_inputs(x, W_qkv, W_out)
    kwargs = {}
    if os.environ.get("BASS_KERNEL_TRACE"):
        kwargs = dict(trace=True, tmpdir=os.environ.get("BASS_KERNEL_TRACE_DIR"))
    res = run_bass_kernel_spmd(nc, in_maps, core_ids=list(range(B * HG)), **kwargs)
    _LAST_RESULTS = res
    out = np.empty((B, T, C), dtype=np.float32)
    for b in range(B):
        out[b] = res.results[b * HG]["y"] + res.results[b * HG + 1]["y"]
    return out


# revision 20
# speedup vs baseline: 1.0103x; 1.0103x over previous
"""Causal self-attention (B=4, T=2048, C=512, H=8, D=64) on 8 TRN2 NeuronCores.

Sharding: core = (batch b, head-group hg) with 4 batches x 2 head groups of 4
heads.  Each core computes q/k/v projections for its 4 heads, causal
attention, and a partial output projection (its 256 rows of W_out); the host
sums the two head-group partials per batch.

Per-core kernel layout notes:
  - x is fed pre-transposed ([C, T]) so all projection matmuls stream
    contiguously (contraction dim = partitions).
  - Attention computes S^T blocks ([tk, tq]) directly by swapping matmul
    operands, so no on-chip transposes are needed anywhere.  tq widths are
    ragged (only tq >= tk is computed), so no masking of above-diagonal
    blocks is needed either -- just a triu mask on the diagonal block.
  - Softmax denominators come from a ones-column appended to V (row 64 of the
    PV accumulator); normalization happens on the [64, 512] O^T eviction via
    a gpsimd partition_broadcast of the reciprocal row.
  - Matmul operands are bf16 (inputs rounded host-side); accumulation and the
    softmax arithmetic stay fp32 in PSUM.
"""

import os
from contextlib import ExitStack

import numpy as np
import ml_dtypes

import concourse.bass as bass
import concourse.tile as tile
from concourse import bacc, mybir
from concourse.bass import ts, ds
from concourse.bass_utils import run_bass_kernel_spmd
from concourse.masks import make_upper_triangular

# Problem constants (hardcoded per harness contract).
B = 4
T = 2048
C = 512
H = 8
D = 64
HG = 2                 # head groups (tensor-parallel dim)
HPC = H // HG          # heads per core = 4
M = HPC * D            # local head width = 256
P = 128
NT = T // P            # 16 t-tiles
NS = T // 512          # 4 t-supertiles
KC = C // P            # 4 contraction chunks of x
F32 = mybir.dt.float32
BF16 = mybir.dt.bfloat16

_LAST_RESULTS = None   # stashed BassKernelResults for test harness inspection


def build_attention_kernel():
    nc = bacc.Bacc("TRN2", target_bir_lowering=False, debug=False, num_devices=B * HG)

    xT = nc.dram_tensor("xT", [C, T], BF16, kind="ExternalInput").ap()
    wq = nc.dram_tensor("wq", [C, M], BF16, kind="ExternalInput").ap()
    wk = nc.dram_tensor("wk", [C, M], BF16, kind="ExternalInput").ap()
    wv = nc.dram_tensor("wv", [C, M], BF16, kind="ExternalInput").ap()
    wo = nc.dram_tensor("wo", [M, C], BF16, kind="ExternalInput").ap()
    y = nc.dram_tensor("y", [T, C], F32, kind="ExternalOutput").ap()

    with tile.TileContext(nc) as tc:
        with ExitStack() as ctx:
            emit_kernel(ctx, tc, xT, wq, wk, wv, wo, y)
    nc.compile()
    return nc


def emit_kernel(ctx, tc, xT, wq, wk, wv, wo, y):
    nc = tc.nc
    Exp = mybir.ActivationFunctionType.Exp
    scale = 1.0 / np.sqrt(D)

    const = ctx.enter_context(tc.tile_pool(name="const", bufs=1))
    xt_pool = ctx.enter_context(tc.tile_pool(name="xt", bufs=1))
    w_pool = ctx.enter_context(tc.tile_pool(name="w", bufs=1))
    qkv_pool = ctx.enter_context(tc.tile_pool(name="qkv", bufs=1))
    pt_pool = ctx.enter_context(tc.tile_pool(name="pt", bufs=6))
    ot_pool = ctx.enter_context(tc.tile_pool(name="ot", bufs=1))
    ysb_pool = ctx.enter_context(tc.tile_pool(name="ysb", bufs=3))
    small_pool = ctx.enter_context(tc.tile_pool(name="small", bufs=4))
    psum_s = ctx.enter_context(tc.tile_pool(name="psum_s", bufs=2, space="PSUM"))
    psum_ot = ctx.enter_context(tc.tile_pool(name="psum_ot", bufs=2, space="PSUM"))

    # --- constants ---
    triu_f32 = const.tile([P, P], F32)
    make_upper_triangular(nc, triu_f32[:], val=1.0, diag=True)
    triu = const.tile([P, P], BF16)
    nc.vector.tensor_copy(triu[:], triu_f32[:])

    # --- PE clock pre-warm: ~4us of dummy back-to-back matmuls during the
    # initial DMA wait flips the HAM clock gate to full rate before real work
    warm_in = const.tile([P, D], BF16)
    nc.gpsimd.memset(warm_in[:], 1.0)
    warm_ps = psum_s.tile([P, 2, 512], F32, name="s_ps")
    for i in range(64):
        nc.tensor.matmul(
            warm_ps[0:D, 0, 0:D], warm_in[:], warm_in[:], start=True, stop=True
        )

    # --- load weights (single strided DMA each) and xT ---
    wq_sb = w_pool.tile([P, KC, M], BF16)
    nc.sync.dma_start(wq_sb[:], wq.rearrange("(kc p) m -> p kc m", p=P))
    wk_sb = w_pool.tile([P, KC, M], BF16)
    nc.sync.dma_start(wk_sb[:], wk.rearrange("(kc p) m -> p kc m", p=P))
    wv_sb = w_pool.tile([P, KC, M], BF16)
    nc.sync.dma_start(wv_sb[:], wv.rearrange("(kc p) m -> p kc m", p=P))
    wo_sb = w_pool.tile([P, M // P, C], BF16)
    nc.sync.dma_start(wo_sb[:], wo.rearrange("(kc p) n -> p kc n", p=P))

    xt_sb = []
    for kc in range(KC):
        t_ = xt_pool.tile([P, T], BF16, name=f"xt{kc}")
        xt_sb.append(t_)
    for tc_ in range(NS):
        for kc in range(KC):
            nc.sync.dma_start(
                xt_sb[kc][:, ts(tc_, 512)], xT[ts(kc, P), ts(tc_, 512)]
            )

    # --- QKV projections ---
    # Q^T, K^T: [M, T] as 2 partition-tiles of [128, T] (2 heads each).
    qt_sb = [qkv_pool.tile([P, T], BF16, name=f"qt{i}") for i in range(M // P)]
    kt_sb = [qkv_pool.tile([P, T], BF16, name=f"kt{i}") for i in range(M // P)]
    # V (+ones col): [128, NT, HPC, D+1]; V block tt rows t in tile, per head.
    v_sb = qkv_pool.tile([P, NT, HPC, D + 1], BF16)
    nc.gpsimd.memset(v_sb[:, :, :, D : D + 1], 1.0)
    def emit_v_block(tt):
        s_ps = psum_s.tile([P, 2, 512], F32, name="s_ps")
        ps = s_ps[:, 0, 0:M]
        for kc in range(KC):
            nc.tensor.matmul(
                ps,
                xt_sb[kc][:, ts(tt, P)],
                wv_sb[:, kc, :],
                start=(kc == 0),
                stop=(kc == KC - 1),
            )
        nc.vector.tensor_copy(
            v_sb[:, tt, :, 0:D], ps.rearrange("p (h d) -> p h d", d=D)
        )

    for tc_ in range(NS):
        for mo in range(M // P):
            for w_sb, dst in ((wq_sb, qt_sb), (wk_sb, kt_sb)):
                s_ps = psum_s.tile([P, 2, 512], F32, name="s_ps")
                ps = s_ps[:, 0, :]
                for kc in range(KC):
                    nc.tensor.matmul(
                        ps,
                        w_sb[:, kc, ts(mo, P)],
                        xt_sb[kc][:, ts(tc_, 512)],
                        start=(kc == 0),
                        stop=(kc == KC - 1),
                    )
                nc.vector.tensor_copy(dst[mo][:, ts(tc_, 512)], ps)
        for tt in range(4 * tc_, 4 * tc_ + 4):
            emit_v_block(tt)

    # V (+ones col): [128, NT, HPC, D+1]; V block tt rows t in tile, per head.

    # --- attention + output projection, supertile by supertile ---
    # Heads are processed in pairs (partition offsets 0 and 64 of one qt/kt
    # tile): the two S^T matmuls are 64x128 row-tiled (T0/T8) and can overlap
    # on the PE array; exp covers both heads in one strided ACT op.
    ot_sb = [ot_pool.tile([P, T], BF16, name=f"ot{i}") for i in range(M // P)]

    def emit_yproj(tt, evict_engine):
        s_ps = psum_s.tile([P, 2, 512], F32, name="s_ps")
        ps = s_ps[:, 0, :]
        for mo in range(M // P):
            nc.tensor.matmul(
                ps,
                ot_sb[mo][:, ts(tt, P)],
                wo_sb[:, mo, :],
                start=(mo == 0),
                stop=(mo == M // P - 1),
            )
        y_sb = ysb_pool.tile([P, C], F32)
        evict_engine.copy(y_sb[:], ps) if evict_engine is nc.scalar \
            else evict_engine.tensor_copy(y_sb[:], ps)
        nc.sync.dma_start(y[ts(tt, P), :], y_sb[:])

    pending_yproj = []
    for s in range(NS):
        nblk = 4 * (s + 1)
        stride = max(1, nblk // 5)
        ot_units = [
            psum_ot.tile([D + 1, 2, 512], F32, name="ot_ps")
            for _ in range(M // P)
        ]
        for j in range(nblk):
            # ragged tq range: only tq >= tk (block-rounded)
            off = max(0, j - 4 * s) * P
            n = 512 - off
            for mo in range(M // P):
                s_ps = psum_s.tile([P, 2, 512], F32, name="s_ps")
                for hs, po in ((0, 0), (1, D)):
                    nc.tensor.matmul(
                        s_ps[:, hs, 0:n],
                        kt_sb[mo][ds(po, D), ts(j, P)],
                        qt_sb[mo][ds(po, D), ds(512 * s + off, n)],
                        start=True,
                        stop=True,
                    )
                pt = pt_pool.tile([P, 2, 512], BF16)
                nc.scalar.activation(
                    pt[:, :, 0:n], s_ps[:, :, 0:n], Exp, scale=scale
                )
                if off > 0 or j == 4 * s:
                    # first 128 cols of the ragged region are the diagonal blk
                    for hs in (0, 1):
                        nc.vector.tensor_mul(
                            pt[:, hs, 0:P], pt[:, hs, 0:P], triu[:]
                        )
                for hs, h in ((0, 2 * mo), (1, 2 * mo + 1)):
                    nc.tensor.matmul(
                        ot_units[mo][:, hs, ds(off, n)],
                        v_sb[:, j, h, :],
                        pt[:, hs, ds(0, n)],
                        start=(j == 0),
                        stop=(j == nblk - 1),
                    )
            # hide the previous supertile's output projections in this
            # supertile's attention stream
            if pending_yproj and j % stride == stride - 1:
                emit_yproj(pending_yproj.pop(0), nc.vector)
        for mo in range(M // P):
            ot_ps = ot_units[mo]
            sums = small_pool.tile([1, 2, 512], F32)
            nc.vector.tensor_copy(sums[:], ot_ps[ds(D, 1), :, :])
            recip = small_pool.tile([1, 2, 512], F32)
            nc.vector.reciprocal_approx_fast(recip[:], sums[:])
            bcast = small_pool.tile([D, 2, 512], F32)
            nc.gpsimd.partition_broadcast(bcast[:], recip[:])
            for hs, po in ((0, 0), (1, D)):
                nc.vector.tensor_mul(
                    ot_sb[mo][ds(po, D), ts(s, 512)],
                    ot_ps[0:D, hs, :],
                    bcast[:, hs, :],
                )
        pending_yproj.extend(range(4 * s, 4 * s + 4))

    # tail: whatever projections are still pending (last supertile's)
    for i, tt in enumerate(pending_yproj):
        emit_yproj(tt, nc.scalar if i % 2 == 0 else nc.vector)


def shard_inputs(x, W_qkv, W_out):
    """Full inputs -> list of 8 per-core input dicts (core = b*HG + hg)."""
    bf16 = ml_dtypes.bfloat16
    x = np.asarray(x, dtype=np.float32)
    W_qkv = np.asarray(W_qkv, dtype=np.float32).astype(bf16)
    W_out = np.asarray(W_out, dtype=np.float32).astype(bf16)
    in_maps = []
    for b in range(B):
        xT = np.ascontiguousarray(x[b].T).astype(bf16)
        for hg in range(HG):
            cols = slice(hg * M, (hg + 1) * M)
            in_maps.append(
                {
                    "xT": xT,
                    "wq": np.ascontiguousarray(W_qkv[:, 0 * C :][:, cols]),
                    "wk": np.ascontiguousarray(W_qkv[:, 1 * C :][:, cols]),
                    "wv": np.ascontiguousarray(W_qkv[:, 2 * C :][:, cols]),
                    "wo": np.ascontiguousarray(W_out[hg * M : (hg + 1) * M, :]),
                }
            )
    return in_maps


_NC_CACHE = None


def kernel(x, W_qkv, W_out):
    global _NC_CACHE, _LAST_RESULTS
    if _NC_CACHE is None:
        _NC_CACHE = build_attention_kernel()
    nc = _NC_CACHE
    in_maps = shard_inputs(x, W_qkv, W_out)
    kwargs = {}
    if os.environ.get("BASS_KERNEL_TRACE"):
        kwargs = dict(trace=True, tmpdir=os.environ.get("BASS_KERNEL_TRACE_DIR"))
    res = run_bass_kernel_spmd(nc, in_maps, core_ids=list(range(B * HG)), **kwargs)
    _LAST_RESULTS = res
    out = np.empty((B, T, C), dtype=np.float32)
    for b in range(B):
        out[b] = res.results[b * HG]["y"] + res.results[b * HG + 1]["y"]
    return out


# revision 21
# speedup vs baseline: 1.0272x; 1.0167x over previous
"""Causal self-attention (B=4, T=2048, C=512, H=8, D=64) on 8 TRN2 NeuronCores.

Sharding: core = (batch b, head-group hg) with 4 batches x 2 head groups of 4
heads.  Each core computes q/k/v projections for its 4 heads, causal
attention, and a partial output projection (its 256 rows of W_out); the host
sums the two head-group partials per batch.

Per-core kernel layout notes:
  - x is fed pre-transposed ([C, T]) so all projection matmuls stream
    contiguously (contraction dim = partitions).
  - Attention computes S^T blocks ([tk, tq]) directly by swapping matmul
    operands, so no on-chip transposes are needed anywhere.  tq widths are
    ragged (only tq >= tk is computed), so no masking of above-diagonal
    blocks is needed either -- just a triu mask on the diagonal block.
  - Softmax denominators come from a ones-column appended to V (row 64 of the
    PV accumulator); normalization happens on the [64, 512] O^T eviction via
    a gpsimd partition_broadcast of the reciprocal row.
  - Matmul operands are bf16 (inputs rounded host-side); accumulation and the
    softmax arithmetic stay fp32 in PSUM.
"""

import os
from contextlib import ExitStack

import numpy as np
import ml_dtypes

import concourse.bass as bass
import concourse.tile as tile
from concourse import bacc, mybir
from concourse.bass import ts, ds
from concourse.bass_utils import run_bass_kernel_spmd
from concourse.masks import make_upper_triangular

# Problem constants (hardcoded per harness contract).
B = 4
T = 2048
C = 512
H = 8
D = 64
HG = 2                 # head groups (tensor-parallel dim)
HPC = H // HG          # heads per core = 4
M = HPC * D            # local head width = 256
P = 128
NT = T // P            # 16 t-tiles
NS = T // 512          # 4 t-supertiles
KC = C // P            # 4 contraction chunks of x
F32 = mybir.dt.float32
BF16 = mybir.dt.bfloat16

_LAST_RESULTS = None   # stashed BassKernelResults for test harness inspection


def build_attention_kernel():
    nc = bacc.Bacc("TRN2", target_bir_lowering=False, debug=False, num_devices=B * HG)

    xT = nc.dram_tensor("xT", [C, T], BF16, kind="ExternalInput").ap()
    wq = nc.dram_tensor("wq", [C, M], BF16, kind="ExternalInput").ap()
    wk = nc.dram_tensor("wk", [C, M], BF16, kind="ExternalInput").ap()
    wv = nc.dram_tensor("wv", [C, M], BF16, kind="ExternalInput").ap()
    wo = nc.dram_tensor("wo", [M, C], BF16, kind="ExternalInput").ap()
    y = nc.dram_tensor("y", [T, C], F32, kind="ExternalOutput").ap()

    with tile.TileContext(nc) as tc:
        with ExitStack() as ctx:
            emit_kernel(ctx, tc, xT, wq, wk, wv, wo, y)
    nc.compile()
    return nc


def emit_kernel(ctx, tc, xT, wq, wk, wv, wo, y):
    nc = tc.nc
    Exp = mybir.ActivationFunctionType.Exp
    scale = 1.0 / np.sqrt(D)

    const = ctx.enter_context(tc.tile_pool(name="const", bufs=1))
    xt_pool = ctx.enter_context(tc.tile_pool(name="xt", bufs=1))
    w_pool = ctx.enter_context(tc.tile_pool(name="w", bufs=1))
    qkv_pool = ctx.enter_context(tc.tile_pool(name="qkv", bufs=1))
    pt_pool = ctx.enter_context(tc.tile_pool(name="pt", bufs=6))
    ot_pool = ctx.enter_context(tc.tile_pool(name="ot", bufs=1))
    ysb_pool = ctx.enter_context(tc.tile_pool(name="ysb", bufs=3))
    small_pool = ctx.enter_context(tc.tile_pool(name="small", bufs=4))
    psum_s = ctx.enter_context(tc.tile_pool(name="psum_s", bufs=2, space="PSUM"))
    psum_ot = ctx.enter_context(tc.tile_pool(name="psum_ot", bufs=2, space="PSUM"))

    # --- constants ---
    triu_f32 = const.tile([P, P], F32)
    make_upper_triangular(nc, triu_f32[:], val=1.0, diag=True)
    triu = const.tile([P, P], BF16)
    nc.vector.tensor_copy(triu[:], triu_f32[:])

    # --- PE clock pre-warm: ~4us of dummy back-to-back matmuls during the
    # initial DMA wait flips the HAM clock gate to full rate before real work
    warm_in = const.tile([P, D], BF16)
    nc.gpsimd.memset(warm_in[:], 1.0)
    warm_ps = psum_s.tile([P, 2, 512], F32, name="s_ps")
    for i in range(64):
        nc.tensor.matmul(
            warm_ps[0:D, 0, 0:D], warm_in[:], warm_in[:], start=True, stop=True
        )

    # --- load weights (single strided DMA each) and xT ---
    wq_sb = w_pool.tile([P, KC, M], BF16)
    nc.sync.dma_start(wq_sb[:], wq.rearrange("(kc p) m -> p kc m", p=P))
    wk_sb = w_pool.tile([P, KC, M], BF16)
    nc.sync.dma_start(wk_sb[:], wk.rearrange("(kc p) m -> p kc m", p=P))
    wv_sb = w_pool.tile([P, KC, M], BF16)
    nc.sync.dma_start(wv_sb[:], wv.rearrange("(kc p) m -> p kc m", p=P))
    wo_sb = w_pool.tile([P, M // P, C], BF16)
    nc.sync.dma_start(wo_sb[:], wo.rearrange("(kc p) n -> p kc n", p=P))

    xt_sb = []
    for kc in range(KC):
        t_ = xt_pool.tile([P, T], BF16, name=f"xt{kc}")
        xt_sb.append(t_)
    for tc_ in range(NS):
        for kc in range(KC):
            nc.sync.dma_start(
                xt_sb[kc][:, ts(tc_, 512)], xT[ts(kc, P), ts(tc_, 512)]
            )

    # --- QKV projections (emitted chunk-by-chunk, interleaved with the
    # attention stream below so the exp pipeline starts early) ---
    qt_sb = [qkv_pool.tile([P, T], BF16, name=f"qt{i}") for i in range(M // P)]
    kt_sb = [qkv_pool.tile([P, T], BF16, name=f"kt{i}") for i in range(M // P)]
    # V (+ones col): [128, NT, HPC, D+1]; V block tt rows t in tile, per head.
    v_sb = qkv_pool.tile([P, NT, HPC, D + 1], BF16)
    nc.gpsimd.memset(v_sb[:, :, :, D : D + 1], 1.0)

    def emit_qk_group(tc_, mo, w_sb, dst):
        s_ps = psum_s.tile([P, 2, 512], F32, name="s_ps")
        ps = s_ps[:, 0, :]
        for kc in range(KC):
            nc.tensor.matmul(
                ps,
                w_sb[:, kc, ts(mo, P)],
                xt_sb[kc][:, ts(tc_, 512)],
                start=(kc == 0),
                stop=(kc == KC - 1),
            )
        nc.vector.tensor_copy(dst[mo][:, ts(tc_, 512)], ps)

    def emit_v_block(tt):
        s_ps = psum_s.tile([P, 2, 512], F32, name="s_ps")
        ps = s_ps[:, 0, 0:M]
        for kc in range(KC):
            nc.tensor.matmul(
                ps,
                xt_sb[kc][:, ts(tt, P)],
                wv_sb[:, kc, :],
                start=(kc == 0),
                stop=(kc == KC - 1),
            )
        nc.vector.tensor_copy(
            v_sb[:, tt, :, 0:D], ps.rearrange("p (h d) -> p h d", d=D)
        )

    def proj_groups_for(tc_):
        groups = []
        for mo in range(M // P):
            for w_sb, dst in ((wq_sb, qt_sb), (wk_sb, kt_sb)):
                groups.append(lambda t=tc_, m=mo, w=w_sb, d=dst: emit_qk_group(t, m, w, d))
        for tt in range(4 * tc_, 4 * tc_ + 4):
            groups.append(lambda t=tt: emit_v_block(t))
        return groups

    # --- attention (+ background projection / output-projection work
    # sprinkled into the PE stream), supertile by supertile ---
    ot_sb = [ot_pool.tile([P, T], BF16, name=f"ot{i}") for i in range(M // P)]

    def emit_yproj(tt, evict_engine):
        s_ps = psum_s.tile([P, 2, 512], F32, name="s_ps")
        ps = s_ps[:, 0, :]
        for mo in range(M // P):
            nc.tensor.matmul(
                ps,
                ot_sb[mo][:, ts(tt, P)],
                wo_sb[:, mo, :],
                start=(mo == 0),
                stop=(mo == M // P - 1),
            )
        y_sb = ysb_pool.tile([P, C], F32)
        evict_engine.copy(y_sb[:], ps) if evict_engine is nc.scalar \
            else evict_engine.tensor_copy(y_sb[:], ps)
        nc.sync.dma_start(y[ts(tt, P), :], y_sb[:])

    for g in proj_groups_for(0):
        g()

    pending_yproj = []
    for s in range(NS):
        nblk = 4 * (s + 1)
        # background PE work to hide inside this supertile's attention:
        # next chunk's projections + previous supertile's output projections
        bg = proj_groups_for(s + 1) if s + 1 < NS else []
        bg += [lambda t=tt: emit_yproj(t, nc.vector) for tt in pending_yproj]
        pending_yproj = list(range(4 * s, 4 * s + 4))
        nslots = nblk * 2
        bg_every = max(1, (nslots + len(bg) - 1) // max(1, len(bg))) if bg else 0
        slot = 0
        ot_units = [
            psum_ot.tile([D + 1, 2, 512], F32, name="ot_ps")
            for _ in range(M // P)
        ]
        for j in range(nblk):
            # ragged tq range: only tq >= tk (block-rounded)
            off = max(0, j - 4 * s) * P
            n = 512 - off
            for mo in range(M // P):
                s_ps = psum_s.tile([P, 2, 512], F32, name="s_ps")
                for hs, po in ((0, 0), (1, D)):
                    nc.tensor.matmul(
                        s_ps[:, hs, 0:n],
                        kt_sb[mo][ds(po, D), ts(j, P)],
                        qt_sb[mo][ds(po, D), ds(512 * s + off, n)],
                        start=True,
                        stop=True,
                    )
                pt = pt_pool.tile([P, 2, 512], BF16)
                nc.scalar.activation(
                    pt[:, :, 0:n], s_ps[:, :, 0:n], Exp, scale=scale
                )
                if off > 0 or j == 4 * s:
                    # first 128 cols of the ragged region are the diagonal blk
                    for hs in (0, 1):
                        nc.vector.tensor_mul(
                            pt[:, hs, 0:P], pt[:, hs, 0:P], triu[:]
                        )
                for hs, h in ((0, 2 * mo), (1, 2 * mo + 1)):
                    nc.tensor.matmul(
                        ot_units[mo][:, hs, ds(off, n)],
                        v_sb[:, j, h, :],
                        pt[:, hs, ds(0, n)],
                        start=(j == 0),
                        stop=(j == nblk - 1),
                    )
                if bg and bg_every and slot % bg_every == bg_every - 1:
                    bg.pop(0)()
                slot += 1
        for g in bg:
            g()
        for mo in range(M // P):
            ot_ps = ot_units[mo]
            sums = small_pool.tile([1, 2, 512], F32)
            nc.vector.tensor_copy(sums[:], ot_ps[ds(D, 1), :, :])
            recip = small_pool.tile([1, 2, 512], F32)
            nc.vector.reciprocal_approx_fast(recip[:], sums[:])
            bcast = small_pool.tile([D, 2, 512], F32)
            nc.gpsimd.partition_broadcast(bcast[:], recip[:])
            for hs, po in ((0, 0), (1, D)):
                nc.vector.tensor_mul(
                    ot_sb[mo][ds(po, D), ts(s, 512)],
                    ot_ps[0:D, hs, :],
                    bcast[:, hs, :],
                )

    # tail: last supertile's output projections
    for i, tt in enumerate(pending_yproj):
        emit_yproj(tt, nc.scalar if i % 2 == 0 else nc.vector)


def shard_inputs(x, W_qkv, W_out):
    """Full inputs -> list of 8 per-core input dicts (core = b*HG + hg)."""
    bf16 = ml_dtypes.bfloat16
    x = np.asarray(x, dtype=np.float32)
    W_qkv = np.asarray(W_qkv, dtype=np.float32).astype(bf16)
    W_out = np.asarray(W_out, dtype=np.float32).astype(bf16)
    in_maps = []
    for b in range(B):
        xT = np.ascontiguousarray(x[b].T).astype(bf16)
        for hg in range(HG):
            cols = slice(hg * M, (hg + 1) * M)
            in_maps.append(
                {
                    "xT": xT,
                    "wq": np.ascontiguousarray(W_qkv[:, 0 * C :][:, cols]),
                    "wk": np.ascontiguousarray(W_qkv[:, 1 * C :][:, cols]),
                    "wv": np.ascontiguousarray(W_qkv[:, 2 * C :][:, cols]),
                    "wo": np.ascontiguousarray(W_out[hg * M : (hg + 1) * M, :]),
                }
            )
    return in_maps


_NC_CACHE = None


def kernel(x, W_qkv, W_out):
    global _NC_CACHE, _LAST_RESULTS
    if _NC_CACHE is None:
        _NC_CACHE = build_attention_kernel()
    nc = _NC_CACHE
    in_maps = shard_inputs(x, W_qkv, W_out)
    kwargs = {}
    if os.environ.get("BASS_KERNEL_TRACE"):
        kwargs = dict(trace=True, tmpdir=os.environ.get("BASS_KERNEL_TRACE_DIR"))
    res = run_bass_kernel_spmd(nc, in_maps, core_ids=list(range(B * HG)), **kwargs)
    _LAST_RESULTS = res
    out = np.empty((B, T, C), dtype=np.float32)
    for b in range(B):
        out[b] = res.results[b * HG]["y"] + res.results[b * HG + 1]["y"]
    return out


# revision 22
# speedup vs baseline: 1.0688x; 1.0405x over previous
"""Causal self-attention (B=4, T=2048, C=512, H=8, D=64) on 8 TRN2 NeuronCores.

Sharding: core = (batch b, head-group hg) with 4 batches x 2 head groups of 4
heads.  Each core computes q/k/v projections for its 4 heads, causal
attention, and a partial output projection (its 256 rows of W_out); the host
sums the two head-group partials per batch.

Per-core kernel layout notes:
  - x is fed pre-transposed ([C, T]) so all projection matmuls stream
    contiguously (contraction dim = partitions).
  - Attention computes S^T blocks ([tk, tq]) directly by swapping matmul
    operands, so no on-chip transposes are needed anywhere.  tq widths are
    ragged (only tq >= tk is computed), so no masking of above-diagonal
    blocks is needed either -- just a triu mask on the diagonal block.
  - Softmax denominators come from a ones-column appended to V (row 64 of the
    PV accumulator); normalization happens on the [64, 512] O^T eviction via
    a gpsimd partition_broadcast of the reciprocal row.
  - Matmul operands are bf16 (inputs rounded host-side); accumulation and the
    softmax arithmetic stay fp32 in PSUM.
"""

import os
from contextlib import ExitStack

import numpy as np
import ml_dtypes

import concourse.bass as bass
import concourse.tile as tile
from concourse import bacc, mybir
from concourse.bass import ts, ds
from concourse.bass_utils import run_bass_kernel_spmd
from concourse.masks import make_upper_triangular

# Problem constants (hardcoded per harness contract).
B = 4
T = 2048
C = 512
H = 8
D = 64
HG = 2                 # head groups (tensor-parallel dim)
HPC = H // HG          # heads per core = 4
M = HPC * D            # local head width = 256
P = 128
NT = T // P            # 16 t-tiles
NS = T // 512          # 4 t-supertiles
KC = C // P            # 4 contraction chunks of x
F32 = mybir.dt.float32
BF16 = mybir.dt.bfloat16

_LAST_RESULTS = None   # stashed BassKernelResults for test harness inspection


def build_attention_kernel():
    nc = bacc.Bacc("TRN2", target_bir_lowering=False, debug=False, num_devices=B * HG)

    xT = nc.dram_tensor("xT", [C, T], BF16, kind="ExternalInput").ap()
    wq = nc.dram_tensor("wq", [C, M], BF16, kind="ExternalInput").ap()
    wk = nc.dram_tensor("wk", [C, M], BF16, kind="ExternalInput").ap()
    wv = nc.dram_tensor("wv", [C, M], BF16, kind="ExternalInput").ap()
    wo = nc.dram_tensor("wo", [M, C], BF16, kind="ExternalInput").ap()
    y = nc.dram_tensor("y", [T, C], F32, kind="ExternalOutput").ap()

    with tile.TileContext(nc) as tc:
        with ExitStack() as ctx:
            emit_kernel(ctx, tc, xT, wq, wk, wv, wo, y)
    nc.compile()
    return nc


def emit_kernel(ctx, tc, xT, wq, wk, wv, wo, y):
    nc = tc.nc
    Exp = mybir.ActivationFunctionType.Exp
    scale = 1.0 / np.sqrt(D)

    const = ctx.enter_context(tc.tile_pool(name="const", bufs=1))
    xt_pool = ctx.enter_context(tc.tile_pool(name="xt", bufs=1))
    w_pool = ctx.enter_context(tc.tile_pool(name="w", bufs=1))
    qkv_pool = ctx.enter_context(tc.tile_pool(name="qkv", bufs=1))
    pt_pool = ctx.enter_context(tc.tile_pool(name="pt", bufs=6))
    ot_pool = ctx.enter_context(tc.tile_pool(name="ot", bufs=1))
    ysb_pool = ctx.enter_context(tc.tile_pool(name="ysb", bufs=3))
    small_pool = ctx.enter_context(tc.tile_pool(name="small", bufs=4))
    psum_s = ctx.enter_context(tc.tile_pool(name="psum_s", bufs=2, space="PSUM"))
    psum_ot = ctx.enter_context(tc.tile_pool(name="psum_ot", bufs=2, space="PSUM"))

    # --- constants ---
    triu_f32 = const.tile([P, P], F32)
    make_upper_triangular(nc, triu_f32[:], val=1.0, diag=True)
    triu = const.tile([P, P], BF16)
    nc.vector.tensor_copy(triu[:], triu_f32[:])

    # --- PE clock pre-warm: ~4us of dummy back-to-back matmuls during the
    # initial DMA wait flips the HAM clock gate to full rate before real work
    warm_in = const.tile([P, D], BF16)
    nc.gpsimd.memset(warm_in[:], 1.0)
    warm_ps = psum_s.tile([P, 2, 512], F32, name="s_ps")
    for i in range(64):
        nc.tensor.matmul(
            warm_ps[0:D, 0, 0:D], warm_in[:], warm_in[:], start=True, stop=True
        )

    # --- load weights and xT; ordered so the first projection chunk's
    # inputs (wq, wk, xt tc0) land first, split across two DMA paths ---
    wq_sb = w_pool.tile([P, KC, M], BF16)
    nc.sync.dma_start(wq_sb[:], wq.rearrange("(kc p) m -> p kc m", p=P))
    wk_sb = w_pool.tile([P, KC, M], BF16)
    nc.gpsimd.dma_start(wk_sb[:], wk.rearrange("(kc p) m -> p kc m", p=P))

    xt_sb = []
    for kc in range(KC):
        t_ = xt_pool.tile([P, T], BF16, name=f"xt{kc}")
        xt_sb.append(t_)

    def load_xt(tc_):
        for kc in range(KC):
            eng = nc.sync if kc % 2 == 0 else nc.gpsimd
            eng.dma_start(
                xt_sb[kc][:, ts(tc_, 512)], xT[ts(kc, P), ts(tc_, 512)]
            )

    load_xt(0)
    wv_sb = w_pool.tile([P, KC, M], BF16)
    nc.sync.dma_start(wv_sb[:], wv.rearrange("(kc p) m -> p kc m", p=P))
    for tc_ in range(1, NS):
        load_xt(tc_)
    wo_sb = w_pool.tile([P, M // P, C], BF16)
    nc.gpsimd.dma_start(wo_sb[:], wo.rearrange("(kc p) n -> p kc n", p=P))

    # --- QKV projections (emitted chunk-by-chunk, interleaved with the
    # attention stream below so the exp pipeline starts early) ---
    qt_sb = [qkv_pool.tile([P, T], BF16, name=f"qt{i}") for i in range(M // P)]
    kt_sb = [qkv_pool.tile([P, T], BF16, name=f"kt{i}") for i in range(M // P)]
    # V (+ones col): [128, NT, HPC, D+1]; V block tt rows t in tile, per head.
    v_sb = qkv_pool.tile([P, NT, HPC, D + 1], BF16)
    nc.gpsimd.memset(v_sb[:, :, :, D : D + 1], 1.0)

    def emit_qk_group(tc_, mo, w_sb, dst):
        s_ps = psum_s.tile([P, 2, 512], F32, name="s_ps")
        ps = s_ps[:, 0, :]
        for kc in range(KC):
            nc.tensor.matmul(
                ps,
                w_sb[:, kc, ts(mo, P)],
                xt_sb[kc][:, ts(tc_, 512)],
                start=(kc == 0),
                stop=(kc == KC - 1),
            )
        nc.vector.tensor_copy(dst[mo][:, ts(tc_, 512)], ps)

    def emit_v_block(tt):
        s_ps = psum_s.tile([P, 2, 512], F32, name="s_ps")
        ps = s_ps[:, 0, 0:M]
        for kc in range(KC):
            nc.tensor.matmul(
                ps,
                xt_sb[kc][:, ts(tt, P)],
                wv_sb[:, kc, :],
                start=(kc == 0),
                stop=(kc == KC - 1),
            )
        nc.vector.tensor_copy(
            v_sb[:, tt, :, 0:D], ps.rearrange("p (h d) -> p h d", d=D)
        )

    def proj_groups_for(tc_):
        groups = []
        for mo in range(M // P):
            for w_sb, dst in ((wq_sb, qt_sb), (wk_sb, kt_sb)):
                groups.append(lambda t=tc_, m=mo, w=w_sb, d=dst: emit_qk_group(t, m, w, d))
        for tt in range(4 * tc_, 4 * tc_ + 4):
            groups.append(lambda t=tt: emit_v_block(t))
        return groups

    # --- attention (+ background projection / output-projection work
    # sprinkled into the PE stream), supertile by supertile ---
    ot_sb = [ot_pool.tile([P, T], BF16, name=f"ot{i}") for i in range(M // P)]

    def emit_yproj(tt, evict_engine):
        s_ps = psum_s.tile([P, 2, 512], F32, name="s_ps")
        ps = s_ps[:, 0, :]
        for mo in range(M // P):
            nc.tensor.matmul(
                ps,
                ot_sb[mo][:, ts(tt, P)],
                wo_sb[:, mo, :],
                start=(mo == 0),
                stop=(mo == M // P - 1),
            )
        y_sb = ysb_pool.tile([P, C], F32)
        evict_engine.copy(y_sb[:], ps) if evict_engine is nc.scalar \
            else evict_engine.tensor_copy(y_sb[:], ps)
        nc.sync.dma_start(y[ts(tt, P), :], y_sb[:])

    for g in proj_groups_for(0):
        g()

    pending_yproj = []
    for s in range(NS):
        nblk = 4 * (s + 1)
        # background PE work to hide inside this supertile's attention:
        # next chunk's projections + previous supertile's output projections
        bg = proj_groups_for(s + 1) if s + 1 < NS else []
        bg += [lambda t=tt: emit_yproj(t, nc.vector) for tt in pending_yproj]
        pending_yproj = list(range(4 * s, 4 * s + 4))
        nslots = nblk * 2
        bg_every = max(1, (nslots + len(bg) - 1) // max(1, len(bg))) if bg else 0
        slot = 0
        ot_units = [
            psum_ot.tile([D + 1, 2, 512], F32, name="ot_ps")
            for _ in range(M // P)
        ]
        for j in range(nblk):
            # ragged tq range: only tq >= tk (block-rounded)
            off = max(0, j - 4 * s) * P
            n = 512 - off
            for mo in range(M // P):
                s_ps = psum_s.tile([P, 2, 512], F32, name="s_ps")
                for hs, po in ((0, 0), (1, D)):
                    nc.tensor.matmul(
                        s_ps[:, hs, 0:n],
                        kt_sb[mo][ds(po, D), ts(j, P)],
                        qt_sb[mo][ds(po, D), ds(512 * s + off, n)],
                        start=True,
                        stop=True,
                    )
                pt = pt_pool.tile([P, 2, 512], BF16)
                nc.scalar.activation(
                    pt[:, :, 0:n], s_ps[:, :, 0:n], Exp, scale=scale
                )
                if off > 0 or j == 4 * s:
                    # first 128 cols of the ragged region are the diagonal blk
                    for hs in (0, 1):
                        nc.vector.tensor_mul(
                            pt[:, hs, 0:P], pt[:, hs, 0:P], triu[:]
                        )
                for hs, h in ((0, 2 * mo), (1, 2 * mo + 1)):
                    nc.tensor.matmul(
                        ot_units[mo][:, hs, ds(off, n)],
                        v_sb[:, j, h, :],
                        pt[:, hs, ds(0, n)],
                        start=(j == 0),
                        stop=(j == nblk - 1),
                    )
                if bg and bg_every and slot % bg_every == bg_every - 1:
                    bg.pop(0)()
                slot += 1
        for g in bg:
            g()
        for mo in range(M // P):
            ot_ps = ot_units[mo]
            sums = small_pool.tile([1, 2, 512], F32)
            nc.scalar.copy(sums[:], ot_ps[ds(D, 1), :, :])
            recip = small_pool.tile([1, 2, 512], F32)
            nc.vector.reciprocal_approx_fast(recip[:], sums[:])
            bcast = small_pool.tile([D, 2, 512], F32)
            nc.gpsimd.partition_broadcast(bcast[:], recip[:])
            for hs, po in ((0, 0), (1, D)):
                nc.vector.tensor_mul(
                    ot_sb[mo][ds(po, D), ts(s, 512)],
                    ot_ps[0:D, hs, :],
                    bcast[:, hs, :],
                )

    # tail: last supertile's output projections
    for i, tt in enumerate(pending_yproj):
        emit_yproj(tt, nc.scalar if i % 2 == 0 else nc.vector)


def shard_inputs(x, W_qkv, W_out):
    """Full inputs -> list of 8 per-core input dicts (core = b*HG + hg)."""
    bf16 = ml_dtypes.bfloat16
    x = np.asarray(x, dtype=np.float32)
    W_qkv = np.asarray(W_qkv, dtype=np.float32).astype(bf16)
    W_out = np.asarray(W_out, dtype=np.float32).astype(bf16)
    in_maps = []
    for b in range(B):
        xT = np.ascontiguousarray(x[b].T).astype(bf16)
        for hg in range(HG):
            cols = slice(hg * M, (hg + 1) * M)
            in_maps.append(
                {
                    "xT": xT,
                    "wq": np.ascontiguousarray(W_qkv[:, 0 * C :][:, cols]),
                    "wk": np.ascontiguousarray(W_qkv[:, 1 * C :][:, cols]),
                    "wv": np.ascontiguousarray(W_qkv[:, 2 * C :][:, cols]),
                    "wo": np.ascontiguousarray(W_out[hg * M : (hg + 1) * M, :]),
                }
            )
    return in_maps


_NC_CACHE = None


def kernel(x, W_qkv, W_out):
    global _NC_CACHE, _LAST_RESULTS
    if _NC_CACHE is None:
        _NC_CACHE = build_attention_kernel()
    nc = _NC_CACHE
    in_maps = shard_inputs(x, W_qkv, W_out)
    kwargs = {}
    if os.environ.get("BASS_KERNEL_TRACE"):
        kwargs = dict(trace=True, tmpdir=os.environ.get("BASS_KERNEL_TRACE_DIR"))
    res = run_bass_kernel_spmd(nc, in_maps, core_ids=list(range(B * HG)), **kwargs)
    _LAST_RESULTS = res
    out = np.empty((B, T, C), dtype=np.float32)
    for b in range(B):
        out[b] = res.results[b * HG]["y"] + res.results[b * HG + 1]["y"]
    return out
